# revision 5
# baseline (speedup 1.0000x reference)
"""Trainium2 Bass kernel v2 for the pre-LN attention block (B=4, N=2048, C=768, H=12).

Sharding: 8 cores = (batch b, query-half qh), as v1. Host rolls tokens for
qh=1 so each core's queries are tokens [0:1024].

v2 changes vs v1:
  - All projections (QKV, out) run as fp8 DoubleRow matmuls: weights and
    activations interleave contraction pairs [K,2,*], halving PE passes.
    Weights are host-scaled by 16 into the fp8 sweet spot; the scale is
    unwound in the PSUM->SBUF casts / exp scale / final output scale.
  - exp writes fp8 es directly in the DoubleRow rhs layout (kt-pair
    interleave), so attention@V is 8 DR matmuls per (head, 512q) instead of
    16 bf16 matmuls at half array width.
  - Softmax denominators still ride as a ones-column in the V lhsT (row 64
    of the O accumulator). Reciprocal uses reciprocal_approx_fast (5x).
  - Residual arrives bf16 with the output bias pre-folded.

On-device layouts (per core):
  z2[kp]   [128, 2, 2048] fp8 — LN output ^T, channel pairs (kp,ko,ki)
  wq2[kp]  [128, 2, 2304] fp8 (x16), wo2[kp] [128, 2, 768] fp8 (x16)
  K^T/Q^T  [128, 2048/1024] bf16 (x4) per head-pair
  v2[ktp]  [128, 12, 2, 80] fp8 — 2x v_true, col 64 = 1.0 (denominator)
  es4[ktp] [128, 2, 2, 512] fp8 — (h, ko, q), DR rhs for attn@V
  o_h      [65, 512] PSUM: rows 0-63 = 2*O, row 64 = den
  ot2[kp]  [128, 2, 1024] fp8 — 16x normalized attention out
  y^T      [768, 1024] fp32 = (psy)/256 + (x^T + b_out)  (psy = 256*attn_proj)
"""

import os
import sys

sys.path.insert(0, "/opt/trn_rl_repo")

import numpy as np
import ml_dtypes

import concourse.bass as bass
import concourse.mybir as mybir
import concourse.tile as tile
from concourse import bacc
from concourse.bass_utils import run_bass_kernel_spmd
from concourse.masks import make_identity


FP32 = mybir.dt.float32
BF16 = mybir.dt.bfloat16
FP8 = mybir.dt.float8e4
AF = mybir.ActivationFunctionType
ALU = mybir.AluOpType
DR = mybir.MatmulPerfMode.DoubleRow

B, N, C, H = 4, 2048, 768, 12
D = C // H            # 64
NQ = N // 2           # 1024 queries per core
P = 128
KT = N // P           # 16 key tiles
KTP = KT // 2         # 8 key-tile pairs
CK = C // P           # 6 contraction tiles
KP = CK // 2          # 3 contraction pair-tiles
NPAIR = H // 2        # 6 head pairs
EPS = 1e-5
SW = 16.0             # fp8 weight scale
SKQ = 4.0             # k/q bf16 scale (psum = SW*raw; kt = psum/4 = 4*true)
EXPS = 1.0 / (8.0 * SKQ * SKQ)   # exp scale: s_psum = 16*s_raw, want exp(s_raw/8)
SV = 2.0              # v2 = 2*v_true
SO = 8.0              # rc = den/8 -> dbs = 8/den -> ot2 = 16*attn_out
YS = 1.0 / (SV * SO * SW)        # psy = wo2^T@ot2 = 16*16*attn_proj -> /256


def build_kernel():
    nc = bacc.Bacc("TRN2", target_bir_lowering=False, debug=False)

    x_nat = nc.dram_tensor("x_nat", [N, C], BF16, kind="ExternalInput").ap()
    xT_res = nc.dram_tensor("xT_res", [C, NQ], BF16, kind="ExternalInput").ap()
    wq2d = nc.dram_tensor("wq2", [KP, P, 2, 3 * C], FP8, kind="ExternalInput").ap()
    bq4 = nc.dram_tensor("bq4", [2 * C], FP32, kind="ExternalInput").ap()
    wo2d = nc.dram_tensor("wo2", [KP, P, 2, C], FP8, kind="ExternalInput").ap()
    yT = nc.dram_tensor("yT", [C, NQ], FP32, kind="ExternalOutput").ap()

    with tile.TileContext(nc) as tc:
        from contextlib import ExitStack
        with ExitStack() as ctx:
            pool = lambda *a, **k: ctx.enter_context(tc.tile_pool(*a, **k))
            const = pool(name="const", bufs=1)
            stats = pool(name="stats", bufs=4)
            xin = pool(name="xin", bufs=KT)
            zbuf = pool(name="zbuf", bufs=3)
            z2p = pool(name="z2", bufs=KP)
            wqp = pool(name="wqp", bufs=KP)
            vp = pool(name="vp", bufs=KTP)
            ktp_pool = pool(name="ktp", bufs=2)
            qtp = pool(name="qtp", bufs=2)
            esp = pool(name="esp", bufs=6)
            otp = pool(name="otp", bufs=NPAIR)
            ot2p = pool(name="ot2p", bufs=KP)
            dnp = pool(name="dnp", bufs=2)
            wop = pool(name="wop", bufs=KP)
            xtp = pool(name="xtp", bufs=CK)
            yst = pool(name="yst", bufs=3)
            ps_acc = pool(name="ps_acc", bufs=4, space="PSUM")
            ps_s = pool(name="ps_s", bufs=2, space="PSUM")

            # ---- constants ----
            ident = const.tile([P, P], BF16, tag="ident")
            make_identity(nc, ident)
            ones64 = const.tile([1, 64], BF16, tag="ones64")
            nc.vector.memset(ones64, 1.0)
            eps_t = const.tile([P, 1], FP32, tag="eps")
            nc.vector.memset(eps_t, EPS)
            rstd_all = const.tile([P, KT], FP32, tag="rstd")
            nmr_all = const.tile([P, KT], FP32, tag="nmr")

            # biases (x4): cols 0-5 q per pair, 6-11 k per pair
            bqqk = const.tile([P, 2 * NPAIR], FP32, tag="bqqk")
            nc.sync.dma_start(
                out=bqqk,
                in_=bass.AP(tensor=bq4.tensor, offset=0, ap=[[1, P], [P, 2 * NPAIR]]))
            bqq = bqqk[:, 0:NPAIR]
            bqk = bqqk[:, NPAIR:2 * NPAIR]

            # x tiles first (startup critical path), then resident weights
            x_t = [xin.tile([P, C], BF16, tag="x", name=f"x{t}") for t in range(KT)]
            for tt in range(KT):
                for g in range(3):
                    nc.gpsimd.dma_start(
                        out=x_t[tt][:, g * 256:(g + 1) * 256],
                        in_=x_nat[tt * P:(tt + 1) * P, g * 256:(g + 1) * 256])
            wq_t = [wqp.tile([P, 2, 3 * C], FP8, tag="wq", name=f"wq{k}") for k in range(KP)]
            for k in range(KP):
                nc.sync.dma_start(out=wq_t[k], in_=wq2d[k])

            # ---- pass 1: LayerNorm statistics per 4-tile group.
            # Odd tiles compute sums on the (otherwise idle) scalar engine via
            # accum_out; even tiles use DVE bn_stats. ----
            muvar = const.tile([P, KT, 2], FP32, tag="muvar")
            mu_all = muvar[:, :, 0]
            var_all = muvar[:, :, 1]
            sd_all = const.tile([P, KT], FP32, tag="sd")
            sums = const.tile([P, KT, 2], FP32, tag="sums")  # [Σx, Σx²] odd tiles
            junk8a = stats.tile([P, C], FP8, tag="junka", bufs=1)
            junk8b = stats.tile([P, C], FP8, tag="junkb", bufs=1)

            def ln_stats(tt):
                xt = x_t[tt]
                if tt % 2 == 0:
                    st = stats.tile([P, 3, 6], FP32, tag="bst")
                    for g in range(3):
                        nc.vector.bn_stats(out=st[:, g, :],
                                           in_=xt[:, g * 256:(g + 1) * 256])
                    nc.vector.bn_aggr(out=muvar[:, tt, :], in_=st)
                else:
                    nc.scalar.activation(out=junk8a, in_=xt, func=AF.Identity,
                                         accum_out=sums[:, tt, 0:1])
                    nc.scalar.activation(out=junk8b, in_=xt, func=AF.Square,
                                         accum_out=sums[:, tt, 1:2])

            def ln_group_tail(tt):
                gs = slice(tt - 3, tt + 1)
                go = slice(tt - 2, tt + 1, 2)  # odd tiles in group
                # mu = Σx/C ; var = Σx²/C - mu²  (odd tiles only)
                nc.vector.tensor_scalar_mul(muvar[:, go, 0], sums[:, go, 0], 1.0 / C)
                nc.vector.tensor_scalar_mul(muvar[:, go, 1], sums[:, go, 1], 1.0 / C)
                sq = stats.tile([P, 2], FP32, tag="sq")
                nc.vector.tensor_mul(sq, muvar[:, go, 0], muvar[:, go, 0])
                nc.vector.scalar_tensor_tensor(out=muvar[:, go, 1], in0=sq,
                                               scalar=-1.0, in1=muvar[:, go, 1],
                                               op0=ALU.mult, op1=ALU.add)
                nc.scalar.activation(out=sd_all[:, gs], in_=var_all[:, gs],
                                     func=AF.Sqrt, bias=eps_t, scale=1.0)
                nc.vector.reciprocal(out=rstd_all[:, gs], in_=sd_all[:, gs])
                nc.vector.tensor_mul(nmr_all[:, gs], mu_all[:, gs],
                                     rstd_all[:, gs])
                nc.vector.tensor_scalar_mul(nmr_all[:, gs], nmr_all[:, gs], -1.0)

            # ---- pass 2: apply LN, transpose to z2 (fp8), project V (DR) ----
            z2 = [z2p.tile([P, 2, N], FP8, tag="z2", name=f"z2_{k}") for k in range(KP)]
            v_t = [vp.tile([P, H, 2, 80], FP8, tag="v", name=f"v{t}") for t in range(KTP)]
            for tp in range(KTP):
                nc.vector.memset(v_t[tp][:, :, :, 64:65], 1.0)

            def v_items(tt):
                # V projection for token tile tt -> v2[tt//2][:, :, tt%2, 0:64]
                items = []
                for off, cw in ((0, 512), (512, 256)):
                    cell = {}
                    for k in range(KP):
                        def mm(tt=tt, off=off, cw=cw, k=k, cell=cell):
                            if k == 0:
                                cell["ps"] = ps_acc.tile([P, 512], FP32,
                                                         tag="acc", name="psv")
                            nc.tensor.matmul(
                                cell["ps"][:, 0:cw],
                                lhsT=z2[k][:, :, tt * P:(tt + 1) * P],
                                rhs=wq_t[k][:, :, 2 * C + off:2 * C + off + cw],
                                start=(k == 0), stop=(k == KP - 1),
                                perf_mode=DR)
                        items.append(mm)
                    def cp(tt=tt, off=off, cw=cw, cell=cell):
                        h0 = off // D
                        nc.vector.tensor_scalar_mul(
                            v_t[tt // 2][:, h0:h0 + cw // D, tt % 2, 0:D],
                            cell["ps"][:, 0:cw].rearrange("p (h d) -> p h d", d=D),
                            SV / SW)
                    items.append(cp)
                return items

            def apply_transpose(tt):
                xt = x_t[tt]
                zt = zbuf.tile([P, C], BF16, tag="z")
                if tt % 2 == 0:
                    nc.vector.tensor_scalar(out=zt, in0=xt,
                                            scalar1=rstd_all[:, tt:tt + 1],
                                            scalar2=nmr_all[:, tt:tt + 1],
                                            op0=ALU.mult, op1=ALU.add)
                else:
                    nc.scalar.activation(out=zt, in_=xt, func=AF.Identity,
                                         scale=rstd_all[:, tt:tt + 1],
                                         bias=nmr_all[:, tt:tt + 1])
                for cb in range(CK):
                    pst = ps_acc.tile([P, P], BF16, tag="acc", name="pst")
                    nc.tensor.transpose(pst, zt[:, cb * P:(cb + 1) * P], ident)
                    nc.vector.tensor_copy(
                        z2[cb // 2][:, cb % 2, tt * P:(tt + 1) * P], pst)

            # ---- per head-pair: K^T, Q^T (DR), attention ----
            def kq_chunk(p, kind, ci, kts, qts):
                items = []
                cell = {}
                for k in range(KP):
                    def mm(kind=kind, ci=ci, k=k, cell=cell, p=p):
                        if k == 0:
                            cell["ps"] = ps_acc.tile([P, 512], FP32,
                                                     tag="acc", name="kqacc")
                        col = C + p * P if kind == "k" else p * P
                        nc.tensor.matmul(
                            cell["ps"][:, 0:512],
                            lhsT=wq_t[k][:, :, col:col + P],
                            rhs=z2[k][:, :, ci * 512:(ci + 1) * 512],
                            start=(k == 0), stop=(k == KP - 1),
                            perf_mode=DR)
                    items.append(mm)
                def bias(kind=kind, ci=ci, cell=cell, p=p):
                    bcol = bqk[:, p:p + 1] if kind == "k" else bqq[:, p:p + 1]
                    dst = kts if kind == "k" else qts
                    nc.vector.tensor_scalar(
                        out=dst[:, ci * 512:(ci + 1) * 512],
                        in0=cell["ps"][:, 0:512],
                        scalar1=1.0 / SW * SKQ, scalar2=bcol,
                        op0=ALU.mult, op1=ALU.add)
                items.append(bias)
                return items

            def kq_items(p, kts, qts):
                items = []
                for kind, ci in [("k", c) for c in range(4)] + \
                                [("q", c) for c in range(2)]:
                    items += kq_chunk(p, kind, ci, kts, qts)
                return items

            ot_sb = [otp.tile([P, NQ], BF16, tag="ot", name=f"ot{p}") for p in range(NPAIR)]
            ot2 = [ot2p.tile([P, 2, NQ], FP8, tag="ot2", name=f"ot2_{k}") for k in range(KP)]
            kt_sb = ktp_pool.tile([P, N], BF16, tag="kt", name="kt0")
            qt_sb = qtp.tile([P, NQ], BF16, tag="qt", name="qt0")

            # ---- fused startup: per 4-tile group, run LN stats -> apply ->
            # transpose -> V proj, then pair-0 K (and Q) projection chunks for
            # the tokens that just became available. ----
            for g in range(4):
                for tt in range(4 * g, 4 * g + 4):
                    ln_stats(tt)
                ln_group_tail(4 * g + 3)
                for tt in range(4 * g, 4 * g + 4):
                    apply_transpose(tt)
                    if tt < KT - 2:
                        for it in v_items(tt):
                            it()
                for it in kq_chunk(0, "k", g, kt_sb, qt_sb):
                    it()
                if g < 2:
                    for it in kq_chunk(0, "q", g, kt_sb, qt_sb):
                        it()
            pending_fin = []
            for p in range(NPAIR):
                if p + 1 < NPAIR:
                    kt_next = ktp_pool.tile([P, N], BF16, tag="kt", name=f"kt{p+1}")
                    qt_next = qtp.tile([P, NQ], BF16, tag="qt", name=f"qt{p+1}")
                    pending = list(kq_items(p + 1, kt_next, qt_next))
                else:
                    kt_next = qt_next = None
                    pending = []
                if p == 0:
                    pending = v_items(KT - 2) + v_items(KT - 1) + pending
                pending.reverse()  # pop() from the front

                def finalize_ch(p, ch, rc):
                    qsl = slice(ch * 512, (ch + 1) * 512)
                    dbc = ps_acc.tile([P, 512], FP32, tag="acc", name="dbc")
                    nc.tensor.matmul(dbc[0:64, 0:512], lhsT=ones64,
                                     rhs=rc[0:1, ch * 512:(ch + 1) * 512],
                                     start=True, stop=True,
                                     tile_position=(0, 0))
                    nc.tensor.matmul(dbc[64:128, 0:512], lhsT=ones64,
                                     rhs=rc[0:1, NQ + ch * 512:NQ + (ch + 1) * 512],
                                     start=True, stop=True,
                                     tile_position=(0, 64))
                    dbs = dnp.tile([P, 512], FP32, tag="dbs", name="dbs")
                    nc.vector.reciprocal_approx_fast(out=dbs, in_=dbc[:, 0:512])
                    nc.vector.tensor_mul(ot2[p // 2][:, p % 2, qsl],
                                         ot_sb[p][:, qsl], dbs)

                def out_proj_items(ch):
                    items = []
                    for o in range(CK):
                        cell = {}
                        qsl = slice(ch * 512, (ch + 1) * 512)
                        for k in range(KP):
                            def mm(o=o, k=k, qsl=qsl, cell=cell):
                                if k == 0:
                                    cell["ps"] = ps_acc.tile([P, 512], FP32,
                                                             tag="acc", name="psy")
                                nc.tensor.matmul(cell["ps"][:, 0:512],
                                                 lhsT=wo_t[k][:, :, o * P:(o + 1) * P],
                                                 rhs=ot2[k][:, :, qsl],
                                                 start=(k == 0), stop=(k == KP - 1),
                                                 perf_mode=DR)
                            items.append(mm)
                        def fin(o=o, qsl=qsl, cell=cell):
                            ys = yst.tile([P, 512], FP32, tag="y")
                            nc.vector.scalar_tensor_tensor(
                                out=ys, in0=cell["ps"][:, 0:512], scalar=float(YS),
                                in1=xr_t[o][:, qsl], op0=ALU.mult, op1=ALU.add)
                            nc.sync.dma_start(out=yT[o * P:(o + 1) * P, qsl], in_=ys)
                        items.append(fin)
                    return items

                rc = dnp.tile([1, 2 * NQ], BF16, tag="recip", name="rc", bufs=1)
                for ch in range(2):
                    qsl = slice(ch * 512, (ch + 1) * 512)
                    o_h = ps_acc.tile([P, 512], FP32, tag="acc", name="o_h")
                    o_h2 = ps_acc.tile([P, 512], FP32, tag="acc", name="o_h2")
                    if p == NPAIR - 1 and ch == 1:
                        pending = list(out_proj_items(0))
                        pending.reverse()
                    attnv_q = []
                    for kt in range(KT):
                        ksl = slice(kt * P, (kt + 1) * P)
                        s_ps = ps_s.tile([P, 1024], FP32, tag="s", name="s_ps")
                        nc.tensor.matmul(s_ps[:, 0:512], lhsT=kt_sb[0:64, ksl],
                                         rhs=qt_sb[0:64, qsl], start=True, stop=True)
                        nc.tensor.matmul(s_ps[:, 512:1024], lhsT=kt_sb[64:128, ksl],
                                         rhs=qt_sb[64:128, qsl], start=True, stop=True)
                        if kt % 2 == 0:
                            es4 = esp.tile([P, 2, 2, 512], FP8, tag="es", name="es4")
                        nc.scalar.activation(
                            out=es4[:, :, kt % 2, :],
                            in_=s_ps.rearrange("p (h q) -> p h q", q=512),
                            func=AF.Exp, scale=float(EXPS))
                        # attn@V for the previous kt pair (issued after this
                        # kt's scores+exp so the scalar engine never waits)
                        while attnv_q:
                            attnv_q.pop()()
                        if kt % 2 == 1:
                            def av(tp=kt // 2, es4=es4, o_h=o_h, o_h2=o_h2, p=p):
                                nc.tensor.matmul(o_h[0:65, 0:512],
                                                 lhsT=v_t[tp][:, 2 * p, :, 0:65],
                                                 rhs=es4[:, 0, :, :],
                                                 start=(tp == 0), stop=(tp == KTP - 1),
                                                 perf_mode=DR)
                                nc.tensor.matmul(o_h2[0:65, 0:512],
                                                 lhsT=v_t[tp][:, 2 * p + 1, :, 0:65],
                                                 rhs=es4[:, 1, :, :],
                                                 start=(tp == 0), stop=(tp == KTP - 1),
                                                 perf_mode=DR)
                            if kt == KT - 1:
                                av()
                            else:
                                attnv_q.append(av)
                        if ch == 0 and kt == 3 and pending_fin:
                            pending_fin.pop(0)()
                        npop = 3 if (p == 0 and ch == 0 and kt < 12) else \
                            (2 if (p == NPAIR - 1 and ch == 1) else 1)
                        for _ in range(npop):
                            if pending:
                                pending.pop()()
                        if pending and (ch * KT + kt) % 3 == 2 and len(pending) > 32 - (ch * KT + kt):
                            pending.pop()()
                    # denominator rows (scaled 1/8) + unnormalized O^T out
                    nc.vector.tensor_scalar_mul(rc[0:1, ch * 512:(ch + 1) * 512],
                                                o_h[D:D + 1, 0:512], 1.0 / SO)
                    nc.vector.tensor_scalar_mul(rc[0:1, NQ + ch * 512:NQ + (ch + 1) * 512],
                                                o_h2[D:D + 1, 0:512], 1.0 / SO)
                    nc.vector.tensor_copy(ot_sb[p][0:64, qsl], o_h[0:64, 0:512])
                    nc.vector.tensor_copy(ot_sb[p][64:128, qsl], o_h2[0:64, 0:512])
                    if p == NPAIR - 1:
                        finalize_ch(p, ch, rc)
                while pending:
                    pending.pop()()
                # normalize deferred into the NEXT pair's matmul stream
                if p < NPAIR - 1:
                    def finalize(p=p, rc=rc):
                        finalize_ch(p, 0, rc)
                        finalize_ch(p, 1, rc)
                    pending_fin.append(finalize)
                kt_sb, qt_sb = kt_next, qt_next
                if p == 1:
                    wo_t = [wop.tile([P, 2, C], FP8, tag="wo", name=f"wo{k}") for k in range(KP)]
                    for k in range(KP):
                        nc.sync.dma_start(out=wo_t[k], in_=wo2d[k])
                if p == 2:
                    xr_t = [xtp.tile([P, NQ], BF16, tag="xr", name=f"xr{o}") for o in range(CK)]
                    for o in range(CK):
                        nc.sync.dma_start(out=xr_t[o], in_=xT_res[o * P:(o + 1) * P, :])
            while pending_fin:
                pending_fin.pop(0)()

            # ---- output projection ch1 (ch0 was overlapped into the last
            # pair's ch1 attention loop) ----
            for it in out_proj_items(1):
                it()

    nc.compile()
    return nc


_NC_CACHE = None


def _prep_in_maps(inputs):
    img = np.asarray(inputs["img_tokens"], dtype=np.float32)
    gamma = np.asarray(inputs["ln_gamma"], dtype=np.float32)
    beta = np.asarray(inputs["ln_beta"], dtype=np.float32)
    w_qkv = np.asarray(inputs["w_qkv"], dtype=np.float32)
    w_out = np.asarray(inputs["w_out"], dtype=np.float32)
    b_out = np.asarray(inputs["b_out"], dtype=np.float32)

    wq_eff = w_qkv * gamma[:, None]
    bq_eff = (beta @ w_qkv).astype(np.float32)
    # V-bias constant across keys -> passes through softmax; fold into out bias
    b_out_eff = (b_out + bq_eff[2 * C:3 * C] @ w_out).astype(np.float32)

    # fp8 DoubleRow weight layouts: row c = kp*256 + ko*128 + ki
    wq8 = np.clip(wq_eff * SW, -240, 240).astype(ml_dtypes.float8_e4m3)
    wq8 = wq8.reshape(KP, 2, P, 3 * C).transpose(0, 2, 1, 3).copy()
    wo8 = np.clip(w_out * SW, -240, 240).astype(ml_dtypes.float8_e4m3)
    wo8 = wo8.reshape(KP, 2, P, C).transpose(0, 2, 1, 3).copy()
    bq4 = (bq_eff[0:2 * C] * SKQ).astype(np.float32)

    in_maps = []
    for c in range(8):
        b, qh = c // 2, c % 2
        if qh == 0:
            x_nat = img[b]
        else:
            x_nat = np.concatenate([img[b, NQ:], img[b, :NQ]], axis=0)
        xr = (img[b, qh * NQ:(qh + 1) * NQ].T + b_out_eff[:, None])
        in_maps.append({
            "x_nat": np.ascontiguousarray(x_nat).astype(ml_dtypes.bfloat16),
            "xT_res": np.ascontiguousarray(xr).astype(ml_dtypes.bfloat16),
            "wq2": wq8,
            "bq4": bq4,
            "wo2": wo8,
        })
    return in_maps


def _assemble(res):
    out = np.zeros((B, N, C), np.float32)
    for c in range(8):
        b, qh = c // 2, c % 2
        out[b, qh * NQ:(qh + 1) * NQ, :] = res.results[c]["yT"].T
    return out


def _get_nc():
    global _NC_CACHE
    if _NC_CACHE is None:
        _NC_CACHE = build_kernel()
    return _NC_CACHE


def kernel(**inputs: np.ndarray) -> np.ndarray:
    res = run_bass_kernel_spmd(_get_nc(), _prep_in_maps(inputs),
                               list(range(8)))
    return _assemble(res)


def run_traced(inputs):
    res = run_bass_kernel_spmd(_get_nc(), _prep_in_maps(inputs),
                               list(range(8)), trace=True)
    return res


if __name__ == "__main__":
    rng = np.random.default_rng(0)
    ins = {
        "img_tokens": rng.standard_normal((B, N, C), dtype=np.float32),
        "ln_gamma": np.ones(C, np.float32),
        "ln_beta": np.zeros(C, np.float32),
        "w_qkv": rng.standard_normal((C, 3 * C), dtype=np.float32) * 0.02,
        "w_out": rng.standard_normal((C, C), dtype=np.float32) * 0.02,
        "b_out": np.zeros(C, np.float32),
    }
    out = kernel(**ins)
    print("out", out.shape, out.dtype)
